# revision 24
# baseline (speedup 1.0000x reference)
"""Multi-head causal attention (B=2, T=2048, C=1024, H=16, HS=64) on 8 TRN2
NeuronCores.

Sharding: batch x head-group. Core c handles batch c//4 and heads
[4*(c%4), 4*(c%4)+4), organized as 2 head-pairs. Each core computes a partial
output [T, C] for its batch (row-shard of w_proj over its 256 contraction
columns); the host sums 4 partials per batch and adds b_proj.

Per-core kernel:
  - All matmul operands are bf16 (fp8 was measured 2-10x over the error
    tolerance); PSUM accumulation is fp32.  The two heads' S^T matmuls
    (contraction K=64 each) are packed into PE row-groups (0,0)/(64,0) via
    tile_position so they run concurrently in the systolic array.
  - Inputs are host-packed so every DMA is one contiguous run per partition,
    ordered by first-need across 3 DMA queues (HBM ~358 GB/s is shared).
  - V_aug[h] [keys, 128]: V (cols 0:64 via pair-level PE transpose of VT,
    4 key blocks per PSUM tile, 2 strided copies) | ones (64:128, memset).
  - Flash-style causal attention in transposed layout, software-pipelined
    one jg-step deep: step jg emits S^T matmuls + ONE exp ACT over a
    [128, 4, 512] PSUM tile (both heads x two key blocks, trimmed to the
    causal column range on diagonal steps), then the O^T matmuls of step
    jg-1, then one independent filler popped from a global deque (QKV for
    later t-groups early, half-proj-chunks for earlier groups late, so the
    final attention group - which has no QKV left - still has PE work).
    The lag keeps the in-order PE queue free of head-of-line stalls behind
    the scalar-engine exp, and the fillers keep the PE HAM clock warm.
  - Normalize with reciprocal_approx_fast; project in bf16 with lhsT=Ohat
    t-chunks accumulated over the two pairs, rhs=w_projT slice.
"""

import math
import sys
from collections import deque
from contextlib import ExitStack

if "/opt/trn_rl_repo" not in sys.path:
    sys.path.insert(0, "/opt/trn_rl_repo")

import numpy as np

import concourse.mybir as mybir
import concourse.tile as tile
from concourse import bacc
from concourse.bass import ts
from concourse.bass_utils import run_bass_kernel_spmd

B, T, C = 2, 2048, 1024
H, HS = 16, 64
NCORES = 8
P = 128
G = 512  # q-group size
NG = T // G
KB = 128  # key block
NPO = C // P  # contraction chunks
F32 = mybir.dt.float32
BF16 = mybir.dt.bfloat16

_nc_cache = {}


def _emit(tc):
    nc = tc.nc
    xt4 = nc.dram_tensor("xt4", [NG, P, NPO, G], BF16, kind="ExternalInput").ap()
    w3 = nc.dram_tensor("w3", [3, P, 2, NPO, 128], BF16, kind="ExternalInput").ap()
    wpt = nc.dram_tensor("wpt", [P, 2, C], BF16, kind="ExternalInput").ap()
    trid = nc.dram_tensor("tri2", [P, 2, P], BF16, kind="ExternalInput").ap()
    identd = nc.dram_tensor("ident", [P, P], BF16, kind="ExternalInput").ap()
    out = nc.dram_tensor("out", [T, C], F32, kind="ExternalOutput").ap()

    ctx = ExitStack()
    persist = ctx.enter_context(tc.tile_pool(name="persist", bufs=1))
    vt_pool = ctx.enter_context(tc.tile_pool(name="vtp", bufs=2))
    pt_pool = ctx.enter_context(tc.tile_pool(name="ptp", bufs=3))
    norm_pool = ctx.enter_context(tc.tile_pool(name="normp", bufs=2))
    out_pool = ctx.enter_context(tc.tile_pool(name="outp", bufs=2))
    st_psum = ctx.enter_context(tc.tile_pool(name="stps", bufs=1, space="PSUM"))
    ot_psum = ctx.enter_context(tc.tile_pool(name="otps", bufs=2, space="PSUM"))
    mm_psum = ctx.enter_context(tc.tile_pool(name="mmps", bufs=2, space="PSUM"))

    wq_sb = persist.tile([P, 2, NPO, 128], BF16, tag="wq")
    wk_sb = persist.tile([P, 2, NPO, 128], BF16, tag="wk")
    wv_sb = persist.tile([P, 2, NPO, 128], BF16, tag="wv")
    wpt_sb = persist.tile([P, 2, C], BF16, tag="wpt")
    tri_sb = persist.tile([P, 2, P], BF16, tag="tri")
    ident = persist.tile([P, P], BF16, tag="ident")
    xts = [persist.tile([P, NPO, G], BF16, tag=f"xt{tg}", name=f"xt{tg}")
           for tg in range(NG)]
    qt = [persist.tile([P, T], BF16, tag=f"qt{p}", name=f"qt{p}") for p in range(2)]
    kt = [persist.tile([P, T], BF16, tag=f"kt{p}", name=f"kt{p}") for p in range(2)]
    ohat = [persist.tile([P, T], BF16, tag=f"oh{p}", name=f"oh{p}") for p in range(2)]
    # per-head V|64s; heads 2*p+hh live in vaug[2*p+hh]
    vaug = [persist.tile([P, T // KB, 128], BF16, tag=f"va{h}", name=f"va{h}")
            for h in range(4)]

    # ---- input loading: one contiguous run per partition, ordered by
    # first-need across 3 queues (they share HBM bandwidth).  Weights and
    # xt0 are split into po-halves so the QKV po-loop's fine-grained region
    # deps let the first matmuls start after only ~0.5MB has landed.
    nc.scalar.dma_start(xts[0][:, 0:4, :], xt4[0][:, 0:4, :])
    nc.sync.dma_start(wq_sb[:], w3[0])
    nc.scalar.dma_start(wk_sb[:], w3[1])
    nc.sync.dma_start(xts[0][:, 4:8, :], xt4[0][:, 4:8, :])
    nc.sync.dma_start(wv_sb[:], w3[2])
    nc.scalar.dma_start(ident[:], identd[:])
    nc.sync.dma_start(tri_sb[:], trid[:])
    nc.sync.dma_start(xts[1][:], xt4[1])
    nc.scalar.dma_start(xts[3][:], xt4[3])
    for h in range(4):
        nc.gpsimd.memset(vaug[h][:, :, 64:128], 1.0)
    nc.gpsimd.dma_start(xts[2][:], xt4[2])
    nc.gpsimd.dma_start(wpt_sb[:], wpt[:])

    def emit_qkv(p, which, tg):
        w_sb = {"q": wq_sb, "k": wk_sb, "v": wv_sb}[which]
        ps = mm_psum.tile([P, G], F32, tag="mm", name=f"qkv{p}{which}{tg}")
        for po in range(NPO):
            nc.tensor.matmul(
                ps[:],
                w_sb[:, p, po, :],
                xts[tg][:, po, :],
                start=(po == 0),
                stop=(po == NPO - 1),
            )
        if which == "q":
            nc.vector.tensor_copy(qt[p][:, ts(tg, G)], ps[:])
        elif which == "k":
            nc.vector.tensor_copy(kt[p][:, ts(tg, G)], ps[:])
        else:
            vt = vt_pool.tile([P, G], BF16, tag="vt", name=f"vt{p}{tg}")
            nc.vector.tensor_copy(vt[:], ps[:])
            trp = mm_psum.tile([P, 4, P], BF16, tag="mm", name=f"tr{p}{tg}")
            for kk in range(4):
                nc.tensor.transpose(trp[:, kk, :], vt[:, ts(kk, P)], ident[:])
            j0 = 4 * tg
            nc.vector.tensor_copy(
                vaug[2 * p][:, j0 : j0 + 4, 0:64], trp[:, :, 0:64]
            )
            nc.vector.tensor_copy(
                vaug[2 * p + 1][:, j0 : j0 + 4, 0:64], trp[:, :, 64:128]
            )

    def emit_attn(p, g, fillers, after_norm_chunk=None):
        qtp, ktp, ohp = qt[p], kt[p], ohat[p]
        l_sb = norm_pool.tile([P, G], F32, tag="lsb", name=f"l{p}{g}")
        rinv = norm_pool.tile([P, G], F32, tag="rinv", name=f"r{p}{g}")
        otps_h = [
            ot_psum.tile([P, G], F32, tag="ot", name=f"ot{p}{g}{h}") for h in range(2)
        ]
        n_j = 4 * g + 4
        steps = n_j // 2
        prev = None
        for jg in range(steps + 1):
            cur = None
            act_ns = 0
            pe_ns = 0
            if jg < steps:
                js = (2 * jg, 2 * jg + 1)
                stps = st_psum.tile([P, 4, G], F32, tag="st", name=f"st{p}{g}{jg}")
                ptt = pt_pool.tile([P, 4, G], BF16, tag="pt", name=f"pt{p}{g}{jg}")
                for idx, j in enumerate(js):
                    r = j - 4 * g
                    # g==0 writes the full q-range so the PSUM slot is fully
                    # initialized before any full-tile exp reads it.
                    q0 = 128 * r if (r > 0 and g > 0) else 0
                    for h in range(2):
                        hb = 64 * h
                        nc.tensor.matmul(
                            stps[:, 2 * idx + h, q0:G],
                            ktp[hb : hb + 64, ts(j, KB)],
                            qtp[hb : hb + 64, G * g + q0 : G * (g + 1)],
                            start=True,
                            stop=True,
                            tile_position=(hb, 0),
                        )
                rmin = 2 * jg - 4 * g
                q0m = 128 * rmin if (rmin > 0 and g > 0) else 0
                act_ns = 4 * (G - q0m) + 293
                for idx, j in enumerate(js):
                    q0s = max(0, 128 * (j - 4 * g))
                    pe_ns += int((G - q0s) / 2.4)  # packed S pair
                nc.scalar.activation(
                    ptt[:, :, q0m:G],
                    stps[:, :, q0m:G],
                    mybir.ActivationFunctionType.Exp,
                    scale=float(HS) ** -0.5,
                )
                for idx, j in enumerate(js):
                    r = j - 4 * g
                    if r >= 0:
                        q0 = 128 * r
                        nc.vector.tensor_tensor(
                            ptt[:, 2 * idx : 2 * idx + 2, q0 : q0 + 128],
                            ptt[:, 2 * idx : 2 * idx + 2, q0 : q0 + 128],
                            tri_sb[:],
                            mybir.AluOpType.mult,
                        )
                cur = (js, ptt)
            if prev is not None:
                for idx, j in enumerate(prev[0]):
                    q0s = max(0, 128 * (j - 4 * g))
                    pe_ns += int(2 * (G - q0s) / 2.4)
            # one filler per step keeps the PE fed behind the exp-dependent
            # O matmuls without starving later steps (measured better than
            # cost-model-based multi-popping in both directions)
            if fillers:
                fillers.popleft()[1]()
            if prev is not None:
                js_p, pt_p = prev
                for idx, j in enumerate(js_p):
                    r = j - 4 * g
                    q0 = 128 * r if r >= 0 else 0
                    for h in range(2):
                        nc.tensor.matmul(
                            otps_h[h][:, q0:G],
                            vaug[2 * p + h][:, j, :],
                            pt_p[:, 2 * idx + h, q0:G],
                            start=(j == 0),
                            stop=(j == n_j - 1),
                        )
            prev = cur
        stag = norm_pool.tile([P, G], F32, tag="stag", name=f"sg{p}{g}")
        for h in range(2):
            hb = 64 * h
            nc.vector.tensor_copy(stag[hb : hb + 64, :], otps_h[h][0:64, :])
            nc.vector.tensor_copy(l_sb[hb : hb + 64, :], otps_h[h][64:128, :])
        nc.vector.reciprocal_approx_fast(rinv[:], l_sb[:])
        if after_norm_chunk is None:
            nc.vector.tensor_tensor(
                ohp[:, ts(g, G)], stag[:], rinv[:], mybir.AluOpType.mult
            )
        else:
            # last call: normalize per 128-query chunk and immediately emit
            # that chunk's projection + output DMA so the drain pipelines
            for tc4 in range(4):
                nc.vector.tensor_tensor(
                    ohp[:, G * g + P * tc4 : G * g + P * (tc4 + 1)],
                    stag[:, ts(tc4, P)],
                    rinv[:, ts(tc4, P)],
                    mybir.AluOpType.mult,
                )
                after_norm_chunk(tc4)

    def make_proj_fillers(g, tc4, last=False):
        t0 = G * g + P * tc4
        cell = {}

        def half(n):
            if n == 0:
                cell["o"] = out_pool.tile(
                    [P, C], F32, tag="osb", name=f"osb{g}{tc4}"
                )
            o_sb = cell["o"]
            pj = mm_psum.tile([P, G], F32, tag="mm", name=f"pj{g}{tc4}{n}")
            for p in range(2):
                nc.tensor.matmul(
                    pj[:],
                    ohat[p][:, t0 : t0 + P],
                    wpt_sb[:, p, ts(n, G)],
                    start=(p == 0),
                    stop=(p == 1),
                )
            if last:
                # final group: split copies across scalar (idle after the
                # last exp) and vector, DMA each half as soon as it lands
                if n == 0:
                    nc.vector.tensor_copy(o_sb[:, ts(n, G)], pj[:])
                else:
                    nc.scalar.copy(o_sb[:, ts(n, G)], pj[:])
                eng = nc.sync if n == 0 else nc.gpsimd
                eng.dma_start(out[t0 : t0 + P, ts(n, G)], o_sb[:, ts(n, G)])
            else:
                nc.vector.tensor_copy(o_sb[:, ts(n, G)], pj[:])
                if n == 1:
                    eng = nc.sync if tc4 % 2 == 0 else nc.gpsimd
                    eng.dma_start(out[t0 : t0 + P, :], o_sb[:])

        return [lambda: half(0), lambda: half(1)]

    # ================= emission =================
    # Lead-in: pair 0's QKV for t-group 0 only; everything else is a filler
    # (cost in ns, thunk).
    QKV_NS, V_NS, PJ_NS = 2000, 3000, 600
    for which in ("q", "k", "v"):
        emit_qkv(0, which, 0)
    fillers = deque()
    for which in ("q", "k", "v"):
        fillers.append(
            (V_NS if which == "v" else QKV_NS, lambda w=which: emit_qkv(1, w, 0))
        )
    for tg in range(1, NG):
        for p in range(2):
            for which in ("q", "k", "v"):
                fillers.append(
                    (V_NS if which == "v" else QKV_NS,
                     lambda p=p, w=which, t=tg: emit_qkv(p, w, t))
                )
    for g in range(NG - 1):
        for tc4 in range(4):
            for f in make_proj_fillers(g, tc4):
                fillers.append((PJ_NS, f))

    def final_proj(tc4):
        for f in make_proj_fillers(NG - 1, tc4, last=True):
            f()

    for g in range(NG):
        emit_attn(0, g, fillers)
        emit_attn(
            1, g, fillers,
            after_norm_chunk=final_proj if g == NG - 1 else None,
        )
    while fillers:
        fillers.popleft()[1]()
    ctx.close()


def _build():
    if "nc" in _nc_cache:
        return _nc_cache["nc"]
    nc = bacc.Bacc("TRN2", target_bir_lowering=False, debug=False)
    with tile.TileContext(nc) as tc:
        _emit(tc)
    nc.compile()
    _nc_cache["nc"] = nc
    return nc


def _make_in_maps(x, wq, wk, wv, w_proj):
    import ml_dtypes

    bf = ml_dtypes.bfloat16
    tri1 = np.triu(np.ones((P, P), dtype=np.float32))
    tri2 = np.ascontiguousarray(np.stack([tri1, tri1], axis=1)).astype(bf)
    ident = np.eye(P, dtype=np.float32).astype(bf)
    # xt4[b][tg, pi, po, t] = x[b, tg*G + t, po*128 + pi]
    xt4 = [
        np.ascontiguousarray(
            x[b].reshape(NG, G, NPO, P).transpose(0, 3, 2, 1)
        ).astype(bf)
        for b in range(B)
    ]

    def pack_w(w, h0):
        # [pi, pair, po, d]: per pair the two heads' [C, 64] slices side by side
        pairs = []
        for pp in range(2):
            wp = np.concatenate([w[h0 + 2 * pp], w[h0 + 2 * pp + 1]], axis=1)
            pairs.append(wp.reshape(NPO, P, P).transpose(1, 0, 2))
        return np.stack(pairs, axis=1)  # [128, 2, NPO, 128]

    in_maps = []
    for c in range(NCORES):
        b, hg = c // 4, c % 4
        h0 = 4 * hg
        w3 = np.ascontiguousarray(
            np.stack([pack_w(wq, h0), pack_w(wk, h0), pack_w(wv, h0)], axis=0)
        ).astype(bf)
        # wpt[pi, p, c] = w_proj[c, 256*hg + 128*p + pi]
        wslice = w_proj[:, 256 * hg : 256 * (hg + 1)].T  # [256, C]
        wpt = np.ascontiguousarray(
            wslice.reshape(2, P, C).transpose(1, 0, 2)
        ).astype(bf)
        in_maps.append(
            {"xt4": xt4[b], "w3": w3, "wpt": wpt, "tri2": tri2, "ident": ident}
        )
    return in_maps


def kernel(x, wq, wk, wv, w_proj, b_proj):
    x = np.asarray(x, dtype=np.float32)
    wq = np.asarray(wq, dtype=np.float32)
    wk = np.asarray(wk, dtype=np.float32)
    wv = np.asarray(wv, dtype=np.float32)
    w_proj = np.asarray(w_proj, dtype=np.float32)
    b_proj = np.asarray(b_proj, dtype=np.float32)

    nc = _build()
    in_maps = _make_in_maps(x, wq, wk, wv, w_proj)
    res = run_bass_kernel_spmd(nc, in_maps, core_ids=list(range(NCORES)))
    acc = np.zeros((B, T, C), dtype=np.float64)
    for c, r in enumerate(res.results):
        acc[c // 4] += r["out"]
    return (acc + b_proj).astype(np.float32)
